# revision 8
# baseline (speedup 1.0000x reference)
"""Bahdanau (additive) attention kernel for Trainium2, 8 NeuronCores.

Reference computation (per batch b):
    inp  = input @ W_in.T + b_in                        # [H]
    ctxp = context @ W_ctx.T + b_ctx                    # [S, H]
    scores[s] = sum_h v[h] * tanh(inp[h] + ctxp[s, h])  # [S]
    scores[mask] = -inf ; attn = softmax(scores)        # [S]
    hidden = sum_s attn[s] * ctxp[s, :]                 # [H]

Strategy:
  - Data-parallel over batch: B=16 over 8 cores, 2 batches/core. No collectives.
  - Host precomputes the small-tensor algebra: bias = input@W_in.T+b_in+b_ctx,
    W_ctx.T layout, v chunking. Device does all O(B*S*D) work.
  - hidden is decomposed: hidden = (attn^T @ context) @ W_ctx.T + b_ctx
    (valid since sum(attn)=1), so the projected ctx never has to be kept.
    Device returns y = attn^T @ context (contraction over the big tensor);
    host applies the tiny [512,512] projection.
  - Device pipeline per batch:
      DMA context natural [s,d] -> PE-transpose tiles to [d,s] -> f32r GEMM
      ctxT[h,s] = W^T^T ctxTr -> fused ACT tanh(x + bias[h]) -> PE v-dot
      -> masked softmax on [8,512] scores -> PE attn-weighted context sum.
"""

import numpy as np
from contextlib import ExitStack

import concourse.bass as bass
import concourse.bacc as bacc
import concourse.tile as tile
from concourse import mybir
from concourse.bass_utils import run_bass_kernel_spmd

B, S, D, H = 16, 4096, 512, 512
NCORES = 8
BPC = B // NCORES          # 2 batches per core
SC = 512                   # s-chunk width (GEMM moving-dim)
NSC = S // SC              # 8 score rows
NT = S // 128              # 32 s-tiles of 128
F32 = mybir.dt.float32
F32R = mybir.dt.float32r
MASK_NEG = -30000.0
AX = mybir.AxisListType.X
AF = mybir.ActivationFunctionType


def _r(ap):
    """View an fp32 AP as float32r for full-rate PE streaming."""
    return ap.bitcast(F32R)


def build_nc(use_f32r=True, f32r_transpose=True):
    nc = bacc.Bacc(None)
    rr = _r if use_f32r else (lambda ap: ap)
    rt = _r if f32r_transpose else (lambda ap: ap)

    ctx_h = nc.dram_tensor("ctx", [BPC, S, D], F32R, kind="ExternalInput")
    wt_h = nc.dram_tensor("wt", [4, 128, H], F32R, kind="ExternalInput")
    v_h = nc.dram_tensor("vv", [128, 4], F32R, kind="ExternalInput")
    inp_h = nc.dram_tensor("inp", [128, BPC * 4], F32, kind="ExternalInput")
    mk_h = nc.dram_tensor("maskf", [BPC, S], F32, kind="ExternalInput")
    eye_h = nc.dram_tensor("eye", [128, 128], F32R, kind="ExternalInput")
    eye8_h = nc.dram_tensor("eye8", [8, 8], F32, kind="ExternalInput")
    attn_h = nc.dram_tensor("attn", [BPC, S], F32, kind="ExternalOutput")
    y_h = nc.dram_tensor("yy", [BPC, D], F32, kind="ExternalOutput")

    with tile.TileContext(nc) as tc, ExitStack() as ctx:
        sing = ctx.enter_context(tc.tile_pool(name="sing", bufs=1))
        ctxp = ctx.enter_context(tc.tile_pool(name="ctxp", bufs=2))
        trp = ctx.enter_context(tc.tile_pool(name="trp", bufs=6))
        thp = ctx.enter_context(tc.tile_pool(name="thp", bufs=3))
        scp = ctx.enter_context(tc.tile_pool(name="scp", bufs=2))
        smp = ctx.enter_context(tc.tile_pool(name="smp", bufs=2))
        psT = ctx.enter_context(tc.tile_pool(name="psT", space="PSUM", bufs=2))
        psHS = ctx.enter_context(tc.tile_pool(name="psHS", space="PSUM", bufs=2))
        psSC = ctx.enter_context(tc.tile_pool(name="psSC", space="PSUM", bufs=2))
        psY = ctx.enter_context(tc.tile_pool(name="psY", space="PSUM", bufs=1))
        psM = ctx.enter_context(tc.tile_pool(name="psM", space="PSUM", bufs=1))

        # --- one-time constant loads ---
        wt_sb = sing.tile([128, 4, H], F32R)
        for i in range(4):
            nc.sync.dma_start(out=wt_sb[:, i, :], in_=wt_h[i])
        v_sb = sing.tile([128, 4], F32R)
        nc.sync.dma_start(out=v_sb, in_=v_h[:])
        inp_sb = sing.tile([128, BPC, 4], F32)
        nc.sync.dma_start(out=inp_sb, in_=inp_h[:].rearrange("p (b c) -> p b c", b=BPC))
        eye_sb = sing.tile([128, 128], F32R)
        nc.sync.dma_start(out=eye_sb, in_=eye_h[:])
        eye8_sb = sing.tile([8, 8], F32)
        nc.sync.dma_start(out=eye8_sb, in_=eye8_h[:])
        ones8r = sing.tile([1, 8], F32)
        nc.vector.memset(ones8r, 1.0)
        nones8r = sing.tile([1, 8], F32)
        nc.vector.memset(nones8r, -1.0)
        ones8c = sing.tile([8, 1], F32)
        nc.vector.memset(ones8c, 1.0)

        for b in range(BPC):
            # --- load this batch's context (natural [s, d] layout) ---
            ctx_nat = ctxp.tile([128, NT, D], F32R, tag="ctxnat")
            for t in range(NT):
                nc.sync.dma_start(
                    out=ctx_nat[:, t, :], in_=ctx_h[b, t * 128 : (t + 1) * 128, :]
                )
            mk_sb = smp.tile([8, SC], F32, tag="mk")
            nc.sync.dma_start(out=mk_sb, in_=mk_h[b].rearrange("(j s) -> j s", j=8))
            scores_sb = smp.tile([8, SC], F32, tag="scores")

            # --- scores pipeline over 8 s-chunks of 512 ---
            for J in range(NSC):
                trs = []
                for i in range(4):
                    pT = psT.tile([128, SC], F32, tag="pT")
                    for c in range(4):
                        t = 4 * J + c
                        nc.tensor.transpose(
                            _r(pT[:, c * 128 : (c + 1) * 128]),
                            ctx_nat[:, t, i * 128 : (i + 1) * 128],
                            eye_sb,
                        )
                    tr = trp.tile([128, SC], F32R, tag="tr")
                    nc.vector.tensor_copy(tr, pT)
                    trs.append(tr)
                pSCt = psSC.tile([1, SC], F32, tag="pSC")
                for h in range(4):
                    pHS = psHS.tile([128, SC], F32, tag="pHS")
                    for i in range(4):
                        nc.tensor.matmul(
                            pHS,
                            wt_sb[:, i, h * 128 : (h + 1) * 128],
                            trs[i],
                            start=(i == 0),
                            stop=(i == 3),
                        )
                    th = thp.tile([128, SC], F32R, tag="th")
                    nc.scalar.activation(
                        th, pHS, AF.Tanh, bias=inp_sb[:, b, h : h + 1], scale=1.0
                    )
                    nc.tensor.matmul(
                        pSCt,
                        v_sb[:, h : h + 1],
                        th,
                        start=(h == 0),
                        stop=(h == 3),
                    )
                # park the row in SBUF, then DMA to partition J
                srow = scp.tile([1, SC], F32, tag="srow")
                nc.vector.tensor_copy(srow, pSCt)
                nc.sync.dma_start(out=scores_sb[J : J + 1, :], in_=srow)

            # --- softmax over [8, 512] = 4096 scores ---
            nc.vector.tensor_add(scores_sb, scores_sb, mk_sb)
            m8 = scp.tile([8, 1], F32, tag="m8")
            nc.vector.reduce_max(m8, scores_sb, axis=AX)
            pMt = psM.tile([128, 8], F32, tag="small")
            pM = pMt[:1, :]
            nc.tensor.matmul(pM, m8, eye8_sb, start=True, stop=True)  # m8^T
            m1 = scp.tile([1, 1], F32, tag="m1")
            nc.vector.reduce_max(m1, pM, axis=AX)
            pBt = psM.tile([128, 8], F32, tag="small")
            pB = pBt[:8, :1]
            nc.tensor.matmul(pB, nones8r, m1, start=True, stop=True)  # bcast -max
            nb8 = scp.tile([8, 1], F32, tag="nb8")
            nc.vector.tensor_copy(nb8, pB)
            p_sb = smp.tile([8, SC], F32, tag="psb")
            sum8 = scp.tile([8, 1], F32, tag="sum8")
            nc.scalar.activation(
                p_sb, scores_sb, AF.Exp, bias=nb8, scale=1.0, accum_out=sum8
            )
            pS1t = psM.tile([128, 8], F32, tag="small")
            pS1 = pS1t[:1, :1]
            nc.tensor.matmul(pS1, sum8, ones8c, start=True, stop=True)  # total
            r1 = scp.tile([1, 1], F32, tag="r1")
            nc.vector.reciprocal(r1, pS1)
            pR8t = psM.tile([128, 8], F32, tag="small")
            pR8 = pR8t[:8, :1]
            nc.tensor.matmul(pR8, ones8r, r1, start=True, stop=True)  # bcast 1/total
            r8 = scp.tile([8, 1], F32, tag="r8")
            nc.vector.tensor_copy(r8, pR8)
            attn_sb = smp.tile([8, SC], F32, tag="attnsb")
            nc.vector.tensor_scalar_mul(attn_sb, p_sb, r8)
            nc.sync.dma_start(out=attn_h[b].rearrange("(j s) -> j s", j=8), in_=attn_sb)

            # --- y = attn^T @ context  (contraction over s on partitions) ---
            atT_sb = scp.tile([128, 4, 8], F32R, tag="atT")
            for c in range(4):
                pA = psM.tile([128, 8], F32, tag="small")
                nc.tensor.matmul(
                    pA,
                    attn_sb[:, c * 128 : (c + 1) * 128],
                    eye8_sb,
                    start=True,
                    stop=True,
                )
                nc.vector.tensor_copy(atT_sb[:, c, :], pA)
            pY = psY.tile([1, D], F32, tag="pY")
            for t in range(NT):
                J, c = t // 4, t % 4
                nc.tensor.matmul(
                    pY,
                    atT_sb[:, c, J : J + 1],
                    ctx_nat[:, t, :],
                    start=(t == 0),
                    stop=(t == NT - 1),
                )
            y_sb = scp.tile([1, D], F32, tag="ysb")
            nc.vector.tensor_copy(y_sb, pY)
            nc.sync.dma_start(out=y_h[b : b + 1, :], in_=y_sb)

    nc.finalize()
    return nc


def _prep_core_inputs(input, context, mask, W_in, b_in, W_ctx, b_ctx, v):
    """Host-side small-tensor prep shared by all cores + per-core slices."""
    input = np.asarray(input, np.float32)
    context = np.ascontiguousarray(np.asarray(context, np.float32))
    mask = np.asarray(mask)
    W_in = np.asarray(W_in, np.float32)
    b_in = np.asarray(b_in, np.float32)
    W_ctx = np.asarray(W_ctx, np.float32)
    b_ctx = np.asarray(b_ctx, np.float32)
    v = np.asarray(v, np.float32)

    bias = input @ W_in.T + b_in + b_ctx  # [B, H] tanh bias incl. b_ctx
    wt_dev = np.ascontiguousarray(W_ctx.T).reshape(4, 128, H)
    v_dev = np.ascontiguousarray(v.reshape(4, 128).T)
    maskf = np.where(mask, np.float32(MASK_NEG), np.float32(0.0)).astype(np.float32)
    eye = np.eye(128, dtype=np.float32)
    eye8 = np.eye(8, dtype=np.float32)

    in_maps = []
    for core in range(NCORES):
        bs = slice(core * BPC, (core + 1) * BPC)
        inp_dev = np.ascontiguousarray(
            bias[bs].reshape(BPC, 4, 128).transpose(2, 0, 1).reshape(128, BPC * 4)
        )
        in_maps.append(
            {
                "ctx": np.ascontiguousarray(context[bs]),
                "wt": wt_dev,
                "vv": v_dev,
                "inp": inp_dev,
                "maskf": np.ascontiguousarray(maskf[bs]),
                "eye": eye,
                "eye8": eye8,
            }
        )
    return in_maps, W_ctx, b_ctx


_NC_CACHE = {}


def _get_nc():
    if "nc" not in _NC_CACHE:
        _NC_CACHE["nc"] = build_nc()
    return _NC_CACHE["nc"]


def kernel(input, context, mask, W_in, b_in, W_ctx, b_ctx, v, _trace=False, **tk):
    in_maps, W_ctx_f, b_ctx_f = _prep_core_inputs(
        input, context, mask, W_in, b_in, W_ctx, b_ctx, v
    )
    nc = _get_nc()
    res = run_bass_kernel_spmd(
        nc, in_maps, core_ids=list(range(NCORES)), trace=_trace, **tk
    )
    attn = np.concatenate([r["attn"] for r in res.results], axis=0)  # [16, 4096]
    y = np.concatenate([r["yy"] for r in res.results], axis=0)  # [16, 512]
    hidden = (y @ W_ctx_f.T + b_ctx_f).astype(np.float32)
    if _trace:
        kernel.last_results = res
    return hidden, attn.astype(np.float32)


# revision 10
# speedup vs baseline: 35.5879x; 35.5879x over previous
"""Bahdanau (additive) attention kernel for Trainium2, 8 NeuronCores.

Reference computation (per batch b):
    inp  = input @ W_in.T + b_in                        # [H]
    ctxp = context @ W_ctx.T + b_ctx                    # [S, H]
    scores[s] = sum_h v[h] * tanh(inp[h] + ctxp[s, h])  # [S]
    scores[mask] = -inf ; attn = softmax(scores)        # [S]
    hidden = sum_s attn[s] * ctxp[s, :]                 # [H]

Strategy:
  - Data-parallel over batch: B=16 over 8 cores, 2 batches/core. No collectives.
  - Host precomputes the small-tensor algebra: bias = input@W_in.T+b_in+b_ctx,
    W_ctx.T layout, v chunking. Device does all O(B*S*D) work.
  - hidden is decomposed: hidden = (attn^T @ context) @ W_ctx.T + b_ctx
    (valid since sum(attn)=1), so the projected ctx never has to be kept.
    Device returns y = attn^T @ context (contraction over the big tensor);
    host applies the tiny [512,512] projection.
  - Device pipeline per batch:
      DMA context natural [s,d] -> PE-transpose tiles to [d,s] -> f32r GEMM
      ctxT[h,s] = W^T^T ctxTr -> fused ACT tanh(x + bias[h]) -> PE v-dot
      -> masked softmax on [8,512] scores -> PE attn-weighted context sum.
"""

import numpy as np
from contextlib import ExitStack

import concourse.bass as bass
import concourse.bacc as bacc
import concourse.tile as tile
from concourse import mybir
from concourse.bass_utils import run_bass_kernel_spmd

B, S, D, H = 16, 4096, 512, 512
NCORES = 8
BPC = B // NCORES          # 2 batches per core
SC = 512                   # s-chunk width (GEMM moving-dim)
NSC = S // SC              # 8 score rows
NT = S // 128              # 32 s-tiles of 128
F32 = mybir.dt.float32
F32R = mybir.dt.float32r
MASK_NEG = -30000.0
AX = mybir.AxisListType.X
AF = mybir.ActivationFunctionType


def _r(ap):
    """View an fp32 AP as float32r for full-rate PE streaming."""
    return ap.bitcast(F32R)


def build_nc(use_f32r=True, f32r_transpose=True, reps=1):
    nc = bacc.Bacc(None)
    rr = _r if use_f32r else (lambda ap: ap)
    rt = _r if f32r_transpose else (lambda ap: ap)

    ctx_h = nc.dram_tensor("ctx", [BPC, S, D], F32R, kind="ExternalInput")
    wt_h = nc.dram_tensor("wt", [4, 128, H], F32R, kind="ExternalInput")
    v_h = nc.dram_tensor("vv", [128, 4], F32R, kind="ExternalInput")
    inp_h = nc.dram_tensor("inp", [128, BPC * 4], F32, kind="ExternalInput")
    mk_h = nc.dram_tensor("maskf", [BPC, S], F32, kind="ExternalInput")
    eye_h = nc.dram_tensor("eye", [128, 128], F32R, kind="ExternalInput")
    eye8_h = nc.dram_tensor("eye8", [8, 8], F32, kind="ExternalInput")
    attn_h = nc.dram_tensor("attn", [BPC, S], F32, kind="ExternalOutput")
    y_h = nc.dram_tensor("yy", [BPC, D], F32, kind="ExternalOutput")

    with tile.TileContext(nc) as tc, ExitStack() as ctx:
        sing = ctx.enter_context(tc.tile_pool(name="sing", bufs=1))
        ctxp = ctx.enter_context(tc.tile_pool(name="ctxp", bufs=2))
        trp = ctx.enter_context(tc.tile_pool(name="trp", bufs=6))
        thp = ctx.enter_context(tc.tile_pool(name="thp", bufs=3))
        scp = ctx.enter_context(tc.tile_pool(name="scp", bufs=2))
        smp = ctx.enter_context(tc.tile_pool(name="smp", bufs=2))
        psT = ctx.enter_context(tc.tile_pool(name="psT", space="PSUM", bufs=2))
        psHS = ctx.enter_context(tc.tile_pool(name="psHS", space="PSUM", bufs=2))
        psSC = ctx.enter_context(tc.tile_pool(name="psSC", space="PSUM", bufs=2))
        psY = ctx.enter_context(tc.tile_pool(name="psY", space="PSUM", bufs=1))
        psM = ctx.enter_context(tc.tile_pool(name="psM", space="PSUM", bufs=1))

        # --- one-time constant loads ---
        wt_sb = sing.tile([128, 4, H], F32R)
        for i in range(4):
            nc.sync.dma_start(out=wt_sb[:, i, :], in_=wt_h[i])
        v_sb = sing.tile([128, 4], F32R)
        nc.sync.dma_start(out=v_sb, in_=v_h[:])
        inp_sb = sing.tile([128, BPC, 4], F32)
        nc.sync.dma_start(out=inp_sb, in_=inp_h[:].rearrange("p (b c) -> p b c", b=BPC))
        eye_sb = sing.tile([128, 128], F32R)
        nc.sync.dma_start(out=eye_sb, in_=eye_h[:])
        eye8_sb = sing.tile([8, 8], F32)
        nc.sync.dma_start(out=eye8_sb, in_=eye8_h[:])
        ones8r = sing.tile([1, 8], F32)
        nc.vector.memset(ones8r, 1.0)
        nones8r = sing.tile([1, 8], F32)
        nc.vector.memset(nones8r, -1.0)
        ones8c = sing.tile([8, 1], F32)
        nc.vector.memset(ones8c, 1.0)

        for _rb in range(reps * BPC):
            b = _rb % BPC
            # --- load this batch's context (natural [s, d] layout) ---
            ctx_nat = ctxp.tile([128, NT, D], F32R, tag="ctxnat")
            for t in range(NT):
                nc.sync.dma_start(
                    out=ctx_nat[:, t, :], in_=ctx_h[b, t * 128 : (t + 1) * 128, :]
                )
            mk_sb = smp.tile([8, SC], F32, tag="mk")
            nc.sync.dma_start(out=mk_sb, in_=mk_h[b].rearrange("(j s) -> j s", j=8))
            scores_sb = smp.tile([8, SC], F32, tag="scores")

            # --- scores pipeline over 8 s-chunks of 512 ---
            for J in range(NSC):
                trs = []
                for i in range(4):
                    pT = psT.tile([128, SC], F32, tag="pT")
                    for c in range(4):
                        t = 4 * J + c
                        nc.tensor.transpose(
                            _r(pT[:, c * 128 : (c + 1) * 128]),
                            ctx_nat[:, t, i * 128 : (i + 1) * 128],
                            eye_sb,
                        )
                    tr = trp.tile([128, SC], F32R, tag="tr")
                    nc.vector.tensor_copy(tr, pT)
                    trs.append(tr)
                pSCt = psSC.tile([1, SC], F32, tag="pSC")
                for h in range(4):
                    pHS = psHS.tile([128, SC], F32, tag="pHS")
                    for i in range(4):
                        nc.tensor.matmul(
                            pHS,
                            wt_sb[:, i, h * 128 : (h + 1) * 128],
                            trs[i],
                            start=(i == 0),
                            stop=(i == 3),
                        )
                    th = thp.tile([128, SC], F32R, tag="th")
                    nc.scalar.activation(
                        th, pHS, AF.Tanh, bias=inp_sb[:, b, h : h + 1], scale=1.0
                    )
                    nc.tensor.matmul(
                        pSCt,
                        v_sb[:, h : h + 1],
                        th,
                        start=(h == 0),
                        stop=(h == 3),
                    )
                # park the row in SBUF, then DMA to partition J
                srow = scp.tile([1, SC], F32, tag="srow")
                nc.vector.tensor_copy(srow, pSCt)
                nc.sync.dma_start(out=scores_sb[J : J + 1, :], in_=srow)

            # --- softmax over [8, 512] = 4096 scores ---
            nc.vector.tensor_add(scores_sb, scores_sb, mk_sb)
            m8 = scp.tile([8, 1], F32, tag="m8")
            nc.vector.reduce_max(m8, scores_sb, axis=AX)
            pMt = psM.tile([128, 8], F32, tag="small")
            pM = pMt[:1, :]
            nc.tensor.matmul(pM, m8, eye8_sb, start=True, stop=True)  # m8^T
            m1 = scp.tile([1, 1], F32, tag="m1")
            nc.vector.reduce_max(m1, pM, axis=AX)
            pBt = psM.tile([128, 8], F32, tag="small")
            pB = pBt[:8, :1]
            nc.tensor.matmul(pB, nones8r, m1, start=True, stop=True)  # bcast -max
            nb8 = scp.tile([8, 1], F32, tag="nb8")
            nc.vector.tensor_copy(nb8, pB)
            p_sb = smp.tile([8, SC], F32, tag="psb")
            sum8 = scp.tile([8, 1], F32, tag="sum8")
            nc.scalar.activation(
                p_sb, scores_sb, AF.Exp, bias=nb8, scale=1.0, accum_out=sum8
            )
            pS1t = psM.tile([128, 8], F32, tag="small")
            pS1 = pS1t[:1, :1]
            nc.tensor.matmul(pS1, sum8, ones8c, start=True, stop=True)  # total
            r1 = scp.tile([1, 1], F32, tag="r1")
            nc.vector.reciprocal(r1, pS1)
            pR8t = psM.tile([128, 8], F32, tag="small")
            pR8 = pR8t[:8, :1]
            nc.tensor.matmul(pR8, ones8r, r1, start=True, stop=True)  # bcast 1/total
            r8 = scp.tile([8, 1], F32, tag="r8")
            nc.vector.tensor_copy(r8, pR8)
            attn_sb = smp.tile([8, SC], F32, tag="attnsb")
            nc.vector.tensor_scalar_mul(attn_sb, p_sb, r8)
            nc.sync.dma_start(out=attn_h[b].rearrange("(j s) -> j s", j=8), in_=attn_sb)

            # --- y = attn^T @ context  (contraction over s on partitions) ---
            atT_sb = scp.tile([128, 4, 8], F32R, tag="atT")
            for c in range(4):
                pA = psM.tile([128, 8], F32, tag="small")
                nc.tensor.matmul(
                    pA,
                    attn_sb[:, c * 128 : (c + 1) * 128],
                    eye8_sb,
                    start=True,
                    stop=True,
                )
                nc.vector.tensor_copy(atT_sb[:, c, :], pA)
            pY = psY.tile([1, D], F32, tag="pY")
            for t in range(NT):
                J, c = t // 4, t % 4
                nc.tensor.matmul(
                    pY,
                    atT_sb[:, c, J : J + 1],
                    ctx_nat[:, t, :],
                    start=(t == 0),
                    stop=(t == NT - 1),
                )
            y_sb = scp.tile([1, D], F32, tag="ysb")
            nc.vector.tensor_copy(y_sb, pY)
            nc.sync.dma_start(out=y_h[b : b + 1, :], in_=y_sb)

    nc.finalize()
    return nc


def _prep_core_inputs(input, context, mask, W_in, b_in, W_ctx, b_ctx, v):
    """Host-side small-tensor prep shared by all cores + per-core slices."""
    input = np.asarray(input, np.float32)
    context = np.ascontiguousarray(np.asarray(context, np.float32))
    mask = np.asarray(mask)
    W_in = np.asarray(W_in, np.float32)
    b_in = np.asarray(b_in, np.float32)
    W_ctx = np.asarray(W_ctx, np.float32)
    b_ctx = np.asarray(b_ctx, np.float32)
    v = np.asarray(v, np.float32)

    bias = input @ W_in.T + b_in + b_ctx  # [B, H] tanh bias incl. b_ctx
    wt_dev = np.ascontiguousarray(W_ctx.T).reshape(4, 128, H)
    v_dev = np.ascontiguousarray(v.reshape(4, 128).T)
    maskf = np.where(mask, np.float32(MASK_NEG), np.float32(0.0)).astype(np.float32)
    eye = np.eye(128, dtype=np.float32)
    eye8 = np.eye(8, dtype=np.float32)

    in_maps = []
    for core in range(NCORES):
        bs = slice(core * BPC, (core + 1) * BPC)
        inp_dev = np.ascontiguousarray(
            bias[bs].reshape(BPC, 4, 128).transpose(2, 0, 1).reshape(128, BPC * 4)
        )
        in_maps.append(
            {
                "ctx": np.ascontiguousarray(context[bs]),
                "wt": wt_dev,
                "vv": v_dev,
                "inp": inp_dev,
                "maskf": np.ascontiguousarray(maskf[bs]),
                "eye": eye,
                "eye8": eye8,
            }
        )
    return in_maps, W_ctx, b_ctx


_NC_CACHE = {}


def _get_nc():
    if "nc" not in _NC_CACHE:
        _NC_CACHE["nc"] = build_nc()
    return _NC_CACHE["nc"]


def kernel(input, context, mask, W_in, b_in, W_ctx, b_ctx, v, _trace=False, **tk):
    in_maps, W_ctx_f, b_ctx_f = _prep_core_inputs(
        input, context, mask, W_in, b_in, W_ctx, b_ctx, v
    )
    nc = _get_nc()
    res = run_bass_kernel_spmd(
        nc, in_maps, core_ids=list(range(NCORES)), trace=_trace, **tk
    )
    attn = np.concatenate([r["attn"] for r in res.results], axis=0)  # [16, 4096]
    y = np.concatenate([r["yy"] for r in res.results], axis=0)  # [16, 512]
    hidden = (y @ W_ctx_f.T + b_ctx_f).astype(np.float32)
    if _trace:
        kernel.last_results = res
    return hidden, attn.astype(np.float32)
